# revision 24
# baseline (speedup 1.0000x reference)
"""Trainium2 Bass kernel for a BailingMoE sparse-MoE block (T=512, H=2048,
E=16 experts top-4 renormalized, expert FFN I=1408, shared expert IS=2816).

v3 strategy (8 NeuronCores, SPMD, no collectives):
  * Expert-parallel: core c owns experts {2c, 2c+1}, processed big-count
    first with ASYMMETRIC capacities (C0 for the bigger expert, C1<=128
    for the smaller), so only one spill segment exists per core.
  * Router on-device in fp32r (measured zero top-4 flips vs fp64 here;
    the host adds +1 capacity for near-tie tokens so either decision
    fits).
  * Routed expert weights are e3m4 fp8 scaled by max/15.5 (halves HBM,
    runs at bf16 rate; dequant scales fold into the sigmoid/evac ops).
    Measured output rel-err ~1.1% vs the 2e-2 tolerance. Shared-expert
    weights and all activations stay bf16; accumulation fp32 in PSUM.
  * Sparse dispatch via one-hot matmuls over a CONCATENATED [T, C0+C1]
    slot space (one MM per (hc, tc) instead of per-expert).
  * Combine matrices Dpw^T are built directly in [slot, T] orientation
    from (idx, weight) columns extracted with tiny matmuls - no PE
    transposes.
  * iota/tri/identity constants shipped from the host; output stored
    bf16 and the host sums the 8 partials in fp32.
"""

import math

import numpy as np
import ml_dtypes

import concourse.bass as bass
import concourse.mybir as mybir
import concourse.tile as tile
from concourse import bacc
from concourse.bass import ts, ds
from concourse.bass_utils import run_bass_kernel_spmd

F32 = mybir.dt.float32
F32R = mybir.dt.float32r
BF16 = mybir.dt.bfloat16
F8E3 = mybir.dt.float8e3
BF = ml_dtypes.bfloat16
E3 = ml_dtypes.float8_e3m4

T, H, E, K, I, IS = 512, 2048, 16, 4, 1408, 2816
NCORES = 8
EPC = E // NCORES            # experts per core
ISC = IS // NCORES           # shared channels per core (352 = 128+128+96)
JS2 = ISC - 256              # third (partial) shared tile width (96)
TT = T // 128                # 4 token tiles
HT = H // 128                # 16 hidden chunks
HK = H // 512                # 4 hidden 512-chunks
IT = I // 128                # 11 expert-intermediate tiles
JSH = 3                      # shared-intermediate tiles per core

AX = mybir.AxisListType
ALU = mybir.AluOpType
ACTF = mybir.ActivationFunctionType


def build_nc(C0: int, C1: int, s_gu: float, s_d: float):
    """SPMD single-core graph; expert 0 capacity C0, expert 1 capacity C1.

    s_gu/s_d: e3m4 quantization scales of the routed gate_up/down weights
    (w8 = w / s); dequant is folded into the sigmoid and y-evac scales.
    """
    caps = [C0, C1]
    off = [0, C0]
    assert C0 % 8 == 0 and C1 % 8 == 0 and 64 <= C1 <= C0 <= 256
    spills = [max(0, c - 128) for c in caps]
    csz1 = sum(spills)
    assert spills[1] == 0 and JS2 + csz1 <= 128
    C2 = C0 + C1                 # concatenated slot space
    SC_Y = float(s_gu) * float(s_gu) * float(s_d)

    nc = bacc.Bacc("TRN2", target_bir_lowering=False, debug=False)

    gw_d = nc.dram_tensor("gw_hl", [128, 2, HT, E], BF16,
                          kind="ExternalInput")
    xtl_d = nc.dram_tensor("xt_lo", [128, HT, T], BF16, kind="ExternalInput")
    xtb_d = nc.dram_tensor("xt_bf", [128, HT, T], BF16, kind="ExternalInput")
    xb_d = nc.dram_tensor("x_bf", [128, TT, H], BF16, kind="ExternalInput")
    cst_d = nc.dram_tensor("csts", [128, T + TT + 128], F32,
                           kind="ExternalInput")   # iota_row | iota_tok | id
    tri_d = nc.dram_tensor("tri", [128, TT, T], BF16, kind="ExternalInput")
    wgu_d = nc.dram_tensor("wgu8", [EPC, IT, 128, 2, H], F8E3,
                           kind="ExternalInput")
    wd_d = nc.dram_tensor("wd8", [EPC, 128, IT, H], F8E3,
                          kind="ExternalInput")
    swgu_d = nc.dram_tensor("swgu", [128, 2, 2, H], BF16,
                            kind="ExternalInput")
    swgu2_d = nc.dram_tensor("swgu2", [128, 2, HT, JS2], BF16,
                             kind="ExternalInput")
    swd_d = nc.dram_tensor("swd", [128, JSH, H], BF16, kind="ExternalInput")
    out_d = nc.dram_tensor("out", [T, H], BF16, kind="ExternalOutput")

    with tile.TileContext(nc) as tc:
        with (
            tc.tile_pool(name="consts", bufs=1) as consts,
            tc.tile_pool(name="persist", bufs=1) as persist,
            tc.tile_pool(name="shp", bufs=1) as shp,
            tc.tile_pool(name="wdp", bufs=1) as wdp,
            tc.tile_pool(name="wpool", bufs=8) as wpool,
            tc.tile_pool(name="rsb", bufs=2) as rsb,
        ):
            cst = consts.tile([128, T + TT + 128], F32)
            iota_row = cst[:, 0:T]
            iota_tok = cst[:, T:T + TT]
            ident_f = cst[:, T + TT:]
            tri_bf = consts.tile([128, TT, T], BF16)

            gw_sb = persist.tile([128, 2, HT, E], BF16)

            wgu_tiles = {}

            def load_wgu(e, j):
                wg = wpool.tile([128, 2, H], F8E3, tag="wgu", name="wgp")
                nc.sync.dma_start(wg, wgu_d[e, j])
                wgu_tiles[(e, j)] = wg

            wd_sb = {}

            def load_wd(e):
                wd_sb[e] = wdp.tile([128, IT, H], F8E3, tag="wd",
                                    name=f"wd{e}")
                for q in range(3):
                    lo = 4 * q
                    n = min(4, IT - lo)
                    nc.sync.dma_start(wd_sb[e][:, ds(lo, n)],
                                      wd_d[e, :, ds(lo, n)])

            # router state (live into the combine phase)
            cw = persist.tile([128, TT, E], F32)
            mask_f = persist.tile([128, TT, E], F32)
            mask_bf = persist.tile([128, TT, E], BF16)
            pos = persist.tile([128, TT, E], F32)
            hsh = persist.tile([128, JSH, T], BF16)
            xd = persist.tile([128, HT, C2], BF16)
            idw0 = {}
            iw1 = None

            # ============ router + shared gate_up + dispatch ============
            with tc.tile_pool(name="actp", bufs=1) as actp:
                x_sb = actp.tile([128, TT, H], BF16, tag="x")
                swgu_sb = shp.tile([128, 2, 2, H], BF16, tag="sw",
                                   name="swgu")
                swgu2_sb = actp.tile([128, 2, HT, JS2], BF16, tag="sw2")
                xt_sb = actp.tile([128, HT, T], BF16, tag="xt")
                D32 = actp.tile([128, TT, C2], F32, tag="D32")
                D16 = actp.tile([128, TT, C2], BF16, tag="D16")

                with (
                    tc.tile_pool(name="pr", space="PSUM", bufs=1) as pr,
                    tc.tile_pool(name="pgsh", space="PSUM", bufs=2) as pgsh,
                ):
                    # FIFO order on the one sync queue: router+shared
                    # hi inputs first, lo correction + the rest after.
                    nc.sync.dma_start(gw_sb, gw_d[:])
                    for q in range(2):
                        nc.sync.dma_start(xt_sb[:, ds(8 * q, 8)],
                                          xtb_d[:, ds(8 * q, 8)])
                    nc.sync.dma_start(swgu_sb[:, 0, 0], swgu_d[:, 0, 0])
                    nc.sync.dma_start(swgu_sb[:, 0, 1], swgu_d[:, 0, 1])
                    xtl_sb = actp.tile([128, HT, T], BF16, tag="xtl")
                    for q in range(2):
                        nc.sync.dma_start(xtl_sb[:, ds(8 * q, 8)],
                                          xtl_d[:, ds(8 * q, 8)])
                    nc.sync.dma_start(swgu_sb[:, 1], swgu_d[:, 1])
                    nc.sync.dma_start(x_sb[:], xb_d[:])
                    nc.sync.dma_start(cst, cst_d[:])
                    nc.sync.dma_start(tri_bf, tri_d[:])
                    nc.sync.dma_start(swgu2_sb[:], swgu2_d[:])
                    # logits = gw_hi.x_hi + gw_lo.x_hi + gw_hi.x_lo
                    lg_ps = pr.tile([16, T], F32, tag="lgT")
                    for hc in range(HT):
                        nc.tensor.matmul(lg_ps, gw_sb[:, 0, hc], xt_sb[:, hc],
                                         start=(hc == 0), stop=False)
                    for hc in range(HT):
                        nc.tensor.matmul(lg_ps, gw_sb[:, 1, hc], xt_sb[:, hc],
                                         start=False, stop=False)
                    for hc in range(HT):
                        nc.tensor.matmul(lg_ps, gw_sb[:, 0, hc],
                                         xtl_sb[:, hc],
                                         start=False, stop=(hc == HT - 1))
                    lgT_sb = rsb.tile([16, T], F32, tag="lgTs")
                    nc.vector.tensor_copy(lgT_sb, lg_ps)

                    lg_all = rsb.tile([128, TT, E], F32, tag="lg_all", bufs=1)
                    for tt in range(TT):
                        lg2 = pr.tile([128, E], F32, tag="small", bufs=2)
                        nc.tensor.transpose(lg2, lgT_sb[:, ts(tt, 128)],
                                            ident_f[:16, :16])
                        nc.vector.tensor_copy(lg_all[:, tt], lg2)

                    def shared_gu(js, pool):
                        P = 128 if js < 2 else JS2
                        ps_g = pool.tile([128, T], F32, tag="gush", name="psg")
                        ps_u = pool.tile([128, T], F32, tag="gush", name="psu")
                        for g, ps in ((0, ps_g), (1, ps_u)):
                            for hc in range(HT):
                                lhsT = (swgu_sb[:, js, g, ts(hc, 128)]
                                        if js < 2 else swgu2_sb[:, g, hc])
                                nc.tensor.matmul(
                                    ps[:P], lhsT, xt_sb[:, hc],
                                    start=(hc == 0), stop=(hc == HT - 1))
                        sg = rsb.tile([128, T], BF16, tag="sgsh", name="sgsh")
                        nc.scalar.activation(sg[:P], ps_g[:P], ACTF.Sigmoid)
                        sg2 = rsb.tile([128, T], BF16, tag="sgsh2",
                                       name="sgsh2")
                        nc.vector.tensor_mul(sg2[:P], sg[:P], ps_g[:P])
                        nc.vector.tensor_mul(hsh[:P, js], sg2[:P], ps_u[:P])
                        if js == 2:
                            # spill rows [96, 96+csz1) are overwritten by
                            # the combine-weight build afterwards
                            nc.vector.memset(hsh[ds(96, 32), 2], 0.0)

                    shared_gu(0, pgsh)
                    shared_gu(1, pgsh)

                    # softmax / top-4 / renormalize (batched)
                    rmn = rsb.tile([128, TT, 1], F32, tag="rmn", bufs=1)
                    nc.vector.tensor_reduce(rmn, lg_all, axis=AX.X,
                                            op=ALU.max, negate=True)
                    lgs = rsb.tile([128, TT, E], F32, tag="lgs", bufs=1)
                    nc.vector.tensor_add(lgs, lg_all,
                                         rmn.to_broadcast([128, TT, E]))
                    ex_all = rsb.tile([128, TT, E], F32, tag="ex_all", bufs=1)
                    nc.scalar.activation(ex_all, lgs, ACTF.Exp)
                    for tt in range(TT):
                        m8 = rsb.tile([128, 8], F32, tag="m8")
                        nc.vector.max(m8, ex_all[:, tt])
                        nc.vector.tensor_scalar(
                            mask_f[:, tt], ex_all[:, tt], m8[:, 3:4], None,
                            op0=ALU.is_ge)

                    cwr = rsb.tile([128, TT, E], F32, tag="cwr", bufs=1)
                    nc.vector.tensor_mul(cwr, ex_all, mask_f[:])
                    s4 = rsb.tile([128, TT, 1], F32, tag="s4", bufs=1)
                    nc.vector.tensor_reduce(s4, cwr, axis=AX.X, op=ALU.add)
                    rs4 = rsb.tile([128, TT, 1], F32, tag="rs4", bufs=1)
                    nc.vector.reciprocal(rs4, s4)
                    nc.vector.tensor_mul(cw[:], cwr,
                                         rs4.to_broadcast([128, TT, E]))
                    nc.vector.tensor_copy(mask_bf[:], mask_f[:])

                    # exclusive cumsum over tokens -> slot positions
                    for tt in range(TT):
                        pos_ps = pr.tile([128, E], F32, tag="small", bufs=2)
                        for tc_ in range(tt + 1):
                            nc.tensor.matmul(
                                pos_ps, tri_bf[:, tc_, ts(tt, 128)],
                                mask_bf[:, tc_], start=(tc_ == 0),
                                stop=(tc_ == tt))
                        nc.vector.tensor_copy(pos[:, tt], pos_ps)

                    # dispatch one-hots over concatenated slot space
                    for e in range(EPC):
                        for tt in range(TT):
                            nc.vector.tensor_scalar(
                                D32[:, tt, ds(off[e], caps[e])],
                                iota_row[:, :caps[e]],
                                pos[:, tt, e:e + 1], mask_f[:, tt, e:e + 1],
                                op0=ALU.is_equal, op1=ALU.mult)
                    nc.vector.tensor_copy(D16[:], D32[:])

                    # (idx, weight) columns per expert via tiny matmuls
                    if csz1 > 0:
                        iw1 = rsb.tile([128, 2], F32, tag="iw1", bufs=1)
                    for e in range(EPC):
                        r2 = rsb.tile([128, TT, 2], F32, tag=f"r2{e}", bufs=1)
                        nc.vector.tensor_copy(r2[:, :, 0], iota_tok)
                        nc.vector.tensor_copy(r2[:, :, 1], cw[:, :, e])
                        iw_ps = pr.tile([128, 2], F32, tag="small", bufs=2)
                        for tc_ in range(TT):
                            nc.tensor.matmul(
                                iw_ps, D32[:, tc_, ds(off[e], 128)],
                                r2[:, tc_], start=(tc_ == 0),
                                stop=(tc_ == TT - 1))
                        iw = rsb.tile([128, 2], F32, tag=f"iw{e}", bufs=1)
                        nc.vector.tensor_copy(iw, iw_ps)
                        idw0[e] = iw
                        if spills[e] > 0:
                            iw1_ps = pr.tile([128, 2], F32, tag="small",
                                             bufs=2)
                            for tc_ in range(TT):
                                nc.tensor.matmul(
                                    iw1_ps[ds(96, spills[e])],
                                    D32[:, tc_, ds(off[e] + 128, spills[e])],
                                    r2[:, tc_], start=(tc_ == 0),
                                    stop=(tc_ == TT - 1),
                                    tile_position=(0, 96))
                            nc.vector.tensor_copy(
                                iw1[ds(96, spills[e])],
                                iw1_ps[ds(96, spills[e])])

                    # dispatch: xd = x^T @ D  -> [h, C0+C1]
                    with tc.tile_pool(name="pd", space="PSUM", bufs=2) as pd:
                        for hc in range(HT):
                            xd_ps = pd.tile([128, C2], F32, tag="xd")
                            for tc_ in range(TT):
                                nc.tensor.matmul(
                                    xd_ps, x_sb[:, tc_, ts(hc, 128)],
                                    D16[:, tc_],
                                    start=(tc_ == 0), stop=(tc_ == TT - 1))
                            if hc == 0:
                                for j in range(2):
                                    load_wgu(0, j)
                            nc.vector.tensor_copy(xd[:, hc], xd_ps)

                    shared_gu(2, pgsh)

                # weighted combine one-hots, built directly transposed
                DpT = {}
                for e in range(EPC):
                    DpT[e] = persist.tile([128, T], BF16, name=f"DpT{e}")
                    nc.vector.tensor_scalar(
                        DpT[e], iota_row, idw0[e][:, 0:1], idw0[e][:, 1:2],
                        op0=ALU.is_equal, op1=ALU.mult)
                if csz1 > 0:
                    # spill combine rows live at partitions 96.. of the
                    # third shared tile (hsh js2 / swd js2)
                    nc.vector.tensor_scalar(
                        hsh[ds(96, csz1), 2],
                        iota_row[ds(96, csz1)],
                        iw1[ds(96, csz1), 0:1],
                        iw1[ds(96, csz1), 1:2],
                        op0=ALU.is_equal, op1=ALU.mult)


            # ============ experts: gate_up -> silu*up -> down ============
            # swd loads now (row range 96.. of its third tile is later
            # overwritten by the spill-down evacuation)
            swd_sb = shp.tile([128, JSH, H], BF16, tag="sw", name="swd")
            nc.sync.dma_start(swd_sb, swd_d[:])
            h_sb = {}
            y0 = {}
            for e in range(EPC):
                Ce = caps[e]
                load_wd(e)
                h_sb[e] = persist.tile([128, IT, Ce], BF16, name=f"h{e}")
                with tc.tile_pool(name=f"pgu{e}", space="PSUM", bufs=6) as pgu:
                    for j in range(IT):
                        if (e, j) in wgu_tiles:
                            wg = wgu_tiles.pop((e, j))
                        else:
                            wg = wpool.tile([128, 2, H], F8E3, tag="wgu")
                            nc.sync.dma_start(wg, wgu_d[e, j])
                        ps_g = pgu.tile([128, Ce], F32, tag="gu")
                        ps_u = pgu.tile([128, Ce], F32, tag="gu")
                        for hc in range(HT):
                            nc.tensor.matmul(
                                ps_g, wg[:, 0, ts(hc, 128)],
                                xd[:, hc, ds(off[e], Ce)],
                                start=(hc == 0), stop=(hc == HT - 1))
                        for hc in range(HT):
                            nc.tensor.matmul(
                                ps_u, wg[:, 1, ts(hc, 128)],
                                xd[:, hc, ds(off[e], Ce)],
                                start=(hc == 0), stop=(hc == HT - 1))
                        # true g = ps_g * s_gu; h kept scaled by 1/s_gu^2
                        sg = rsb.tile([128, Ce], BF16, tag="sg")
                        nc.scalar.activation(sg, ps_g, ACTF.Sigmoid,
                                             scale=float(s_gu))
                        sg2 = rsb.tile([128, Ce], BF16, tag="sg2")
                        nc.vector.tensor_mul(sg2, sg, ps_g)
                        nc.vector.tensor_mul(h_sb[e][:, j], sg2, ps_u)
                        if e + 1 < EPC and j >= IT - 2:
                            load_wgu(e + 1, j - (IT - 2))

                # down ct0: full-array MMs, N=512
                y0[e] = persist.tile([128, H], BF16, name=f"y0{e}")
                with tc.tile_pool(name=f"py{e}", space="PSUM", bufs=1) as py:
                    ps_y = [py.tile([128, 512], F32, tag=f"py{hk}",
                                    name=f"psy{hk}") for hk in range(HK)]
                    for ic in range(IT):
                        for hk in range(HK):
                            nc.tensor.matmul(
                                ps_y[hk], h_sb[e][:, ic, 0:128],
                                wd_sb[e][:, ic, ts(hk, 512)],
                                start=(ic == 0), stop=(ic == IT - 1))
                    for hk in range(HK):
                        if hk % 2 == 0:
                            nc.vector.tensor_scalar(
                                y0[e][:, ts(hk, 512)], ps_y[hk], SC_Y, None,
                                op0=ALU.mult)
                        else:
                            nc.scalar.activation(
                                y0[e][:, ts(hk, 512)], ps_y[hk], ACTF.Copy,
                                scale=SC_Y)

                # down spill slots (only expert 0 can spill): psum and
                # evac live at partitions 96.. so they land in the third
                # shared tile rows of swd
                if spills[e] > 0:
                    with tc.tile_pool(name=f"pm{e}", space="PSUM",
                                      bufs=1) as pm:
                        ps_m = [pm.tile([128, 512], F32, tag=f"pm{hk}",
                                        name=f"pm{e}{hk}")
                                for hk in range(HK)]
                        sl = ds(96, spills[e])
                        for ic in range(IT):
                            for hk in range(HK):
                                nc.tensor.matmul(
                                    ps_m[hk][sl],
                                    h_sb[e][:, ic, ds(128, spills[e])],
                                    wd_sb[e][:, ic, ts(hk, 512)],
                                    start=(ic == 0), stop=(ic == IT - 1),
                                    tile_position=(0, 96))
                        for hk in range(HK):
                            if hk % 2 == 0:
                                nc.vector.tensor_scalar(
                                    swd_sb[sl, 2, ts(hk, 512)], ps_m[hk][sl],
                                    SC_Y, None, op0=ALU.mult)
                            else:
                                nc.scalar.activation(
                                    swd_sb[sl, 2, ts(hk, 512)], ps_m[hk][sl],
                                    ACTF.Copy, scale=SC_Y)

            # ------------- combine: routed (weighted) + shared -------------
            with (
                tc.tile_pool(name="po", space="PSUM", bufs=3) as po,
                tc.tile_pool(name="osb", bufs=4) as osb,
            ):
                chain = [("sh", js) for js in range(JSH)]
                chain += [("e0", e) for e in range(EPC)]
                for tt in range(TT):
                    for hk in range(HK):
                        ps_o = po.tile([128, 512], F32, tag="o", name="ps_o")
                        for n, (a, b) in enumerate(chain):
                            st, sp = (n == 0), (n == len(chain) - 1)
                            if a == "sh":
                                nc.tensor.matmul(
                                    ps_o, hsh[:, b, ts(tt, 128)],
                                    swd_sb[:, b, ts(hk, 512)],
                                    start=st, stop=sp)
                            else:
                                nc.tensor.matmul(
                                    ps_o, DpT[b][:, ts(tt, 128)],
                                    y0[b][:, ts(hk, 512)], start=st, stop=sp)
                        o_sb = osb.tile([128, 512], BF16, tag="o")
                        if hk % 2 == 0:
                            nc.vector.tensor_copy(o_sb, ps_o)
                        else:
                            nc.scalar.copy(o_sb, ps_o)
                        nc.sync.dma_start(
                            out_d[ts(tt, 128), ts(hk, 512)], o_sb)
    nc.compile()
    return nc


def _route(x64: np.ndarray, gate_w64: np.ndarray):
    """fp64 routing + near-tie margin; pairs the i-th biggest-count expert
    with the i-th smallest on each core. Returns (C0, C1, pairs)."""
    logits = x64 @ gate_w64.T
    order = np.argsort(-logits, axis=1)
    counts = np.bincount(order[:, :K].ravel(), minlength=E).astype(np.int64)
    srt = np.sort(logits, axis=1)
    gap = srt[:, E - K] - srt[:, E - K - 1]
    for t in np.nonzero(gap < 2e-3)[0]:
        counts[order[t, K - 1]] += 1
        counts[order[t, K]] += 1
    by_count = sorted(range(E), key=lambda e: -counts[e])
    pairs = [(by_count[c], by_count[E - 1 - c]) for c in range(NCORES)]
    r8 = lambda v: ((int(v) + 7) // 8) * 8
    C0 = min(256, max(128, r8(max(counts[a] for a, _ in pairs))))
    C1 = min(C0, max(64, r8(max(counts[b] for _, b in pairs))))
    return C0, C1, pairs


def _lhsT_tiles(Wt: np.ndarray, col0: int, ncol: int = 128) -> np.ndarray:
    """Wt: [H, cols]. Returns [128, HT*ncol] with (p, ncol*hc + c) =
    Wt[128*hc + p, col0 + c] - lhsT chunk layout over the 16 h-chunks."""
    blk = Wt[:, col0:col0 + ncol].reshape(HT, 128, ncol)
    return np.ascontiguousarray(blk.transpose(1, 0, 2)).reshape(128, HT * ncol)


_BUILD_CACHE = {}


def prepare(hidden_states, gate_w, w_gate_up, w_down, shared_gate_up,
            shared_down):
    """Host-side sharding/layout/quantization prep.

    Returns (C0, C1, s_gu, s_d, in_maps)."""
    x = np.ascontiguousarray(np.asarray(hidden_states, dtype=np.float32))
    gate_w = np.asarray(gate_w, dtype=np.float32)
    w_gate_up = np.asarray(w_gate_up, dtype=np.float32)
    w_down = np.asarray(w_down, dtype=np.float32)
    shared_gate_up = np.asarray(shared_gate_up, dtype=np.float32)
    shared_down = np.asarray(shared_down, dtype=np.float32)

    C0, C1, pairs = _route(x.astype(np.float64), gate_w.astype(np.float64))
    s_gu = float(np.abs(w_gate_up).max()) / 15.5
    s_d = float(np.abs(w_down).max()) / 15.5

    # common (replicated) activations + constants
    xt_r = np.ascontiguousarray(
        x.T.reshape(HT, 128, T).transpose(1, 0, 2))     # [128, HT, T] f32
    xt_bf = xt_r.astype(BF)
    xt_lo = (xt_r - xt_bf.astype(np.float32)).astype(BF)
    x_bf = np.ascontiguousarray(
        x.reshape(TT, 128, H).transpose(1, 0, 2)).astype(BF)
    csts = np.zeros((128, T + TT + 128), np.float32)
    csts[:, 0:T] = np.arange(T, dtype=np.float32)[None, :]
    csts[:, T:T + TT] = (np.arange(128, dtype=np.float32)[:, None]
                         + 128.0 * np.arange(TT, dtype=np.float32)[None, :])
    csts[:, T + TT:] = np.eye(128, dtype=np.float32)
    tri = np.ascontiguousarray(
        np.triu(np.ones((T, T), np.float32), 1)
        .reshape(TT, 128, T).transpose(1, 0, 2)).astype(BF)

    # shared expert: per-core shard of gate/up/down (352 channels each)
    sg_T = np.ascontiguousarray(shared_gate_up[:IS].T)
    su_T = np.ascontiguousarray(shared_gate_up[IS:].T)
    sd_T = np.ascontiguousarray(shared_down.T)

    in_maps = []
    for c in range(NCORES):
        own = list(pairs[c])                     # big-count expert first
        perm = own + [e for e in range(E) if e not in own]
        gperm = gate_w[perm].T.reshape(HT, 128, E).transpose(1, 0, 2)
        gw_hl = np.empty((128, 2, HT, E), BF)
        gw_hl[:, 0] = gperm.astype(BF)
        gw_hl[:, 1] = (gperm - gw_hl[:, 0].astype(np.float32)).astype(BF)

        wgu8 = np.empty((EPC, IT, 128, 2, H), E3)
        wd8 = np.empty((EPC, 128, IT, H), E3)
        for el, eg in enumerate(own):
            Wt8f = (w_gate_up[eg].T / s_gu).astype(E3).astype(np.float32)
            for j in range(IT):
                wgu8[el, j, :, 0, :] = _lhsT_tiles(Wt8f, j * 128).astype(E3)
                wgu8[el, j, :, 1, :] = _lhsT_tiles(Wt8f, I + j * 128).astype(E3)
            Wd8 = (w_down[eg].T / s_d).astype(E3)            # [I, H]
            wd8[el] = Wd8.reshape(IT, 128, H).transpose(1, 0, 2)

        base = c * ISC
        swgu = np.empty((128, 2, 2, H), BF)
        for js in range(2):
            swgu[:, js, 0, :] = _lhsT_tiles(sg_T, base + js * 128).astype(BF)
            swgu[:, js, 1, :] = _lhsT_tiles(su_T, base + js * 128).astype(BF)
        swgu2 = np.empty((128, 2, HT * JS2), BF)
        swgu2[:, 0] = _lhsT_tiles(sg_T, base + 256, JS2).astype(BF)
        swgu2[:, 1] = _lhsT_tiles(su_T, base + 256, JS2).astype(BF)
        swgu2 = swgu2.reshape(128, 2, HT, JS2)
        swd = np.zeros((128, JSH, H), BF)
        swd[:, 0] = sd_T[base:base + 128].astype(BF)
        swd[:, 1] = sd_T[base + 128:base + 256].astype(BF)
        swd[:JS2, 2] = sd_T[base + 256:base + ISC].astype(BF)

        in_maps.append({
            "gw_hl": gw_hl, "xt_lo": xt_lo, "xt_bf": xt_bf, "x_bf": x_bf,
            "csts": csts, "tri": tri,
            "wgu8": wgu8, "wd8": wd8, "swgu": swgu, "swgu2": swgu2,
            "swd": swd,
        })
    return C0, C1, s_gu, s_d, in_maps


def kernel(**inputs):
    C0, C1, s_gu, s_d, in_maps = prepare(**inputs)
    key = (C0, C1, round(s_gu, 9), round(s_d, 9))
    if key not in _BUILD_CACHE:
        _BUILD_CACHE[key] = build_nc(C0, C1, s_gu, s_d)
    nc = _BUILD_CACHE[key]

    res = run_bass_kernel_spmd(nc, in_maps, core_ids=list(range(NCORES)))
    out = np.zeros((T, H), np.float32)
    for r in res.results:
        out += r["out"].astype(np.float32)
    return out


# revision 25
# speedup vs baseline: 1.1923x; 1.1923x over previous
"""Trainium2 Bass kernel for a BailingMoE sparse-MoE block (T=512, H=2048,
E=16 experts top-4 renormalized, expert FFN I=1408, shared expert IS=2816).

v3 strategy (8 NeuronCores, SPMD, no collectives):
  * Expert-parallel: core c owns experts {2c, 2c+1}, processed big-count
    first with ASYMMETRIC capacities (C0 for the bigger expert, C1<=128
    for the smaller), so only one spill segment exists per core.
  * Router on-device in fp32r (measured zero top-4 flips vs fp64 here;
    the host adds +1 capacity for near-tie tokens so either decision
    fits).
  * Routed expert weights are e3m4 fp8 scaled by max/15.5 (halves HBM,
    runs at bf16 rate; dequant scales fold into the sigmoid/evac ops).
    Measured output rel-err ~1.1% vs the 2e-2 tolerance. Shared-expert
    weights and all activations stay bf16; accumulation fp32 in PSUM.
  * Sparse dispatch via one-hot matmuls over a CONCATENATED [T, C0+C1]
    slot space (one MM per (hc, tc) instead of per-expert).
  * Combine matrices Dpw^T are built directly in [slot, T] orientation
    from (idx, weight) columns extracted with tiny matmuls - no PE
    transposes.
  * iota/tri/identity constants shipped from the host; output stored
    bf16 and the host sums the 8 partials in fp32.
"""

import numpy as np
import ml_dtypes

import concourse.mybir as mybir
import concourse.tile as tile
from concourse import bacc
from concourse.bass import ts, ds
from concourse.bass_utils import run_bass_kernel_spmd

F32 = mybir.dt.float32
F32R = mybir.dt.float32r
BF16 = mybir.dt.bfloat16
F8E3 = mybir.dt.float8e3
BF = ml_dtypes.bfloat16
E3 = ml_dtypes.float8_e3m4

T, H, E, K, I, IS = 512, 2048, 16, 4, 1408, 2816
NCORES = 8
EPC = E // NCORES            # experts per core
ISC = IS // NCORES           # shared channels per core (352 = 128+128+96)
JS2 = ISC - 256              # third (partial) shared tile width (96)
TT = T // 128                # 4 token tiles
HT = H // 128                # 16 hidden chunks
HK = H // 512                # 4 hidden 512-chunks
IT = I // 128                # 11 expert-intermediate tiles
JSH = 3                      # shared-intermediate tiles per core

AX = mybir.AxisListType
ALU = mybir.AluOpType
ACTF = mybir.ActivationFunctionType


def build_nc(C0: int, C1: int, s_gu: float, s_d: float):
    """SPMD single-core graph; expert 0 capacity C0, expert 1 capacity C1.

    s_gu/s_d: e3m4 quantization scales of the routed gate_up/down weights
    (w8 = w / s); dequant is folded into the sigmoid and y-evac scales.
    """
    caps = [C0, C1]
    off = [0, C0]
    assert C0 % 8 == 0 and C1 % 8 == 0 and 64 <= C1 <= C0 <= 256
    spills = [max(0, c - 128) for c in caps]
    csz1 = sum(spills)
    assert spills[1] == 0 and JS2 + csz1 <= 128
    C2 = C0 + C1                 # concatenated slot space
    SC_Y = float(s_gu) * float(s_gu) * float(s_d)

    nc = bacc.Bacc("TRN2", target_bir_lowering=False, debug=False)

    gw_d = nc.dram_tensor("gw_hl", [128, 2, HT, E], BF16,
                          kind="ExternalInput")
    xtl_d = nc.dram_tensor("xt_lo", [128, HT, T], BF16, kind="ExternalInput")
    xtb_d = nc.dram_tensor("xt_bf", [128, HT, T], BF16, kind="ExternalInput")
    xb_d = nc.dram_tensor("x_bf", [128, TT, H], BF16, kind="ExternalInput")
    cst_d = nc.dram_tensor("csts", [128, T + TT + 128], F32,
                           kind="ExternalInput")   # iota_row | iota_tok | id
    tri_d = nc.dram_tensor("tri", [128, TT, T], BF16, kind="ExternalInput")
    wgu_d = nc.dram_tensor("wgu8", [EPC, IT, 128, 2, H], F8E3,
                           kind="ExternalInput")
    wd_d = nc.dram_tensor("wd8", [EPC, 128, IT, H], F8E3,
                          kind="ExternalInput")
    swgu_d = nc.dram_tensor("swgu", [128, 2, 2, H], BF16,
                            kind="ExternalInput")
    swgu2_d = nc.dram_tensor("swgu2", [128, 2, HT, JS2], BF16,
                             kind="ExternalInput")
    swd_d = nc.dram_tensor("swd", [128, JSH, H], BF16, kind="ExternalInput")
    out_d = nc.dram_tensor("out", [T, H], BF16, kind="ExternalOutput")

    with tile.TileContext(nc) as tc:
        with (
            tc.tile_pool(name="consts", bufs=1) as consts,
            tc.tile_pool(name="persist", bufs=1) as persist,
            tc.tile_pool(name="shp", bufs=1) as shp,
            tc.tile_pool(name="wdp", bufs=1) as wdp,
            tc.tile_pool(name="wpool", bufs=8) as wpool,
            tc.tile_pool(name="rsb", bufs=2) as rsb,
        ):
            cst = consts.tile([128, T + TT + 128], F32)
            iota_row = cst[:, 0:T]
            iota_tok = cst[:, T:T + TT]
            ident_f = cst[:, T + TT:]
            tri_bf = consts.tile([128, TT, T], BF16)

            gw_sb = persist.tile([128, 2, HT, E], BF16)

            wgu_tiles = {}

            def load_wgu(e, j):
                wg = wpool.tile([128, 2, H], F8E3, tag="wgu", name="wgp")
                nc.sync.dma_start(wg, wgu_d[e, j])
                wgu_tiles[(e, j)] = wg

            wd_sb = {}

            def load_wd(e):
                wd_sb[e] = wdp.tile([128, IT, H], F8E3, tag="wd",
                                    name=f"wd{e}")
                for q in range(3):
                    lo = 4 * q
                    n = min(4, IT - lo)
                    nc.sync.dma_start(wd_sb[e][:, ds(lo, n)],
                                      wd_d[e, :, ds(lo, n)])

            # router state (live into the combine phase)
            cw = persist.tile([128, TT, E], F32)
            mask_f = persist.tile([128, TT, E], F32)
            mask_bf = persist.tile([128, TT, E], BF16)
            pos = persist.tile([128, TT, E], F32)
            hsh = persist.tile([128, JSH, T], BF16)
            xd = persist.tile([128, HT, C2], BF16)
            idw0 = {}
            iw1 = None

            # ============ router + shared gate_up + dispatch ============
            with tc.tile_pool(name="actp", bufs=1) as actp:
                x_sb = actp.tile([128, TT, H], BF16, tag="x")
                swgu_sb = shp.tile([128, 2, 2, H], BF16, tag="sw",
                                   name="swgu")
                swgu2_sb = actp.tile([128, 2, HT, JS2], BF16, tag="sw2")
                xt_sb = actp.tile([128, HT, T], BF16, tag="xt")
                D32 = actp.tile([128, TT, C2], F32, tag="D32")
                D16 = actp.tile([128, TT, C2], BF16, tag="D16")

                with (
                    tc.tile_pool(name="pr", space="PSUM", bufs=1) as pr,
                    tc.tile_pool(name="pgsh", space="PSUM", bufs=2) as pgsh,
                ):
                    # FIFO order on the one sync queue: router+shared
                    # hi inputs first, lo correction + the rest after.
                    nc.sync.dma_start(gw_sb, gw_d[:])
                    for q in range(2):
                        nc.sync.dma_start(xt_sb[:, ds(8 * q, 8)],
                                          xtb_d[:, ds(8 * q, 8)])
                    nc.sync.dma_start(swgu_sb[:, 0, 0], swgu_d[:, 0, 0])
                    nc.sync.dma_start(swgu_sb[:, 0, 1], swgu_d[:, 0, 1])
                    xtl_sb = actp.tile([128, HT, T], BF16, tag="xtl")
                    for q in range(2):
                        nc.sync.dma_start(xtl_sb[:, ds(8 * q, 8)],
                                          xtl_d[:, ds(8 * q, 8)])
                    nc.sync.dma_start(swgu_sb[:, 1], swgu_d[:, 1])
                    nc.sync.dma_start(x_sb[:], xb_d[:])
                    nc.sync.dma_start(cst, cst_d[:])
                    nc.sync.dma_start(tri_bf, tri_d[:])
                    nc.sync.dma_start(swgu2_sb[:], swgu2_d[:])
                    # logits = gw_hi.x_hi + gw_lo.x_hi + gw_hi.x_lo
                    lg_ps = pr.tile([16, T], F32, tag="lgT")
                    for hc in range(HT):
                        nc.tensor.matmul(lg_ps, gw_sb[:, 0, hc], xt_sb[:, hc],
                                         start=(hc == 0), stop=False)
                    for hc in range(HT):
                        nc.tensor.matmul(lg_ps, gw_sb[:, 1, hc], xt_sb[:, hc],
                                         start=False, stop=False)
                    for hc in range(HT):
                        nc.tensor.matmul(lg_ps, gw_sb[:, 0, hc],
                                         xtl_sb[:, hc],
                                         start=False, stop=(hc == HT - 1))
                    lgT_sb = rsb.tile([16, T], F32, tag="lgTs")
                    nc.vector.tensor_copy(lgT_sb, lg_ps)

                    lg_all = rsb.tile([128, TT, E], F32, tag="lg_all", bufs=1)
                    for tt in range(TT):
                        lg2 = pr.tile([128, E], F32, tag="small", bufs=2)
                        nc.tensor.transpose(lg2, lgT_sb[:, ts(tt, 128)],
                                            ident_f[:16, :16])
                        nc.vector.tensor_copy(lg_all[:, tt], lg2)

                    def shared_gu(js, pool):
                        P = 128 if js < 2 else JS2
                        ps_g = pool.tile([128, T], F32, tag="gush", name="psg")
                        ps_u = pool.tile([128, T], F32, tag="gush", name="psu")
                        for g, ps in ((0, ps_g), (1, ps_u)):
                            for hc in range(HT):
                                lhsT = (swgu_sb[:, js, g, ts(hc, 128)]
                                        if js < 2 else swgu2_sb[:, g, hc])
                                nc.tensor.matmul(
                                    ps[:P], lhsT, xt_sb[:, hc],
                                    start=(hc == 0), stop=(hc == HT - 1))
                        sg = rsb.tile([128, T], BF16, tag="sgsh", name="sgsh")
                        nc.scalar.activation(sg[:P], ps_g[:P], ACTF.Sigmoid)
                        sg2 = rsb.tile([128, T], BF16, tag="sgsh2",
                                       name="sgsh2")
                        nc.vector.tensor_mul(sg2[:P], sg[:P], ps_g[:P])
                        nc.vector.tensor_mul(hsh[:P, js], sg2[:P], ps_u[:P])
                        if js == 2:
                            # spill rows [96, 96+csz1) are overwritten by
                            # the combine-weight build afterwards
                            nc.vector.memset(hsh[ds(96, 32), 2], 0.0)

                    shared_gu(0, pgsh)
                    shared_gu(1, pgsh)

                    # softmax / top-4 / renormalize (batched)
                    rmn = rsb.tile([128, TT, 1], F32, tag="rmn", bufs=1)
                    nc.vector.tensor_reduce(rmn, lg_all, axis=AX.X,
                                            op=ALU.max, negate=True)
                    lgs = rsb.tile([128, TT, E], F32, tag="lgs", bufs=1)
                    nc.vector.tensor_add(lgs, lg_all,
                                         rmn.to_broadcast([128, TT, E]))
                    ex_all = rsb.tile([128, TT, E], F32, tag="ex_all", bufs=1)
                    nc.scalar.activation(ex_all, lgs, ACTF.Exp)
                    for tt in range(TT):
                        m8 = rsb.tile([128, 8], F32, tag="m8")
                        nc.vector.max(m8, ex_all[:, tt])
                        nc.vector.tensor_scalar(
                            mask_f[:, tt], ex_all[:, tt], m8[:, 3:4], None,
                            op0=ALU.is_ge)

                    cwr = rsb.tile([128, TT, E], F32, tag="cwr", bufs=1)
                    nc.vector.tensor_mul(cwr, ex_all, mask_f[:])
                    s4 = rsb.tile([128, TT, 1], F32, tag="s4", bufs=1)
                    nc.vector.tensor_reduce(s4, cwr, axis=AX.X, op=ALU.add)
                    rs4 = rsb.tile([128, TT, 1], F32, tag="rs4", bufs=1)
                    nc.vector.reciprocal(rs4, s4)
                    nc.vector.tensor_mul(cw[:], cwr,
                                         rs4.to_broadcast([128, TT, E]))
                    nc.vector.tensor_copy(mask_bf[:], mask_f[:])

                    # exclusive cumsum over tokens -> slot positions
                    for tt in range(TT):
                        pos_ps = pr.tile([128, E], F32, tag="small", bufs=2)
                        for tc_ in range(tt + 1):
                            nc.tensor.matmul(
                                pos_ps, tri_bf[:, tc_, ts(tt, 128)],
                                mask_bf[:, tc_], start=(tc_ == 0),
                                stop=(tc_ == tt))
                        nc.vector.tensor_copy(pos[:, tt], pos_ps)

                    # dispatch one-hots over concatenated slot space
                    for e in range(EPC):
                        for tt in range(TT):
                            nc.vector.tensor_scalar(
                                D32[:, tt, ds(off[e], caps[e])],
                                iota_row[:, :caps[e]],
                                pos[:, tt, e:e + 1], mask_f[:, tt, e:e + 1],
                                op0=ALU.is_equal, op1=ALU.mult)
                    nc.vector.tensor_copy(D16[:], D32[:])

                    # (idx, weight) columns per expert via tiny matmuls
                    if csz1 > 0:
                        iw1 = rsb.tile([128, 2], F32, tag="iw1", bufs=1)
                    for e in range(EPC):
                        r2 = rsb.tile([128, TT, 2], F32, tag=f"r2{e}", bufs=1)
                        nc.vector.tensor_copy(r2[:, :, 0], iota_tok)
                        nc.vector.tensor_copy(r2[:, :, 1], cw[:, :, e])
                        iw_ps = pr.tile([128, 2], F32, tag="small", bufs=2)
                        for tc_ in range(TT):
                            nc.tensor.matmul(
                                iw_ps, D32[:, tc_, ds(off[e], 128)],
                                r2[:, tc_], start=(tc_ == 0),
                                stop=(tc_ == TT - 1))
                        iw = rsb.tile([128, 2], F32, tag=f"iw{e}", bufs=1)
                        nc.vector.tensor_copy(iw, iw_ps)
                        idw0[e] = iw
                        if spills[e] > 0:
                            iw1_ps = pr.tile([128, 2], F32, tag="small",
                                             bufs=2)
                            for tc_ in range(TT):
                                nc.tensor.matmul(
                                    iw1_ps[ds(96, spills[e])],
                                    D32[:, tc_, ds(off[e] + 128, spills[e])],
                                    r2[:, tc_], start=(tc_ == 0),
                                    stop=(tc_ == TT - 1),
                                    tile_position=(0, 96))
                            nc.vector.tensor_copy(
                                iw1[ds(96, spills[e])],
                                iw1_ps[ds(96, spills[e])])

                    # dispatch: xd = x^T @ D  -> [h, C0+C1]
                    with tc.tile_pool(name="pd", space="PSUM", bufs=2) as pd:
                        for hc in range(HT):
                            xd_ps = pd.tile([128, C2], F32, tag="xd")
                            for tc_ in range(TT):
                                nc.tensor.matmul(
                                    xd_ps, x_sb[:, tc_, ts(hc, 128)],
                                    D16[:, tc_],
                                    start=(tc_ == 0), stop=(tc_ == TT - 1))
                            if hc == 0:
                                for j in range(2):
                                    load_wgu(0, j)
                            nc.vector.tensor_copy(xd[:, hc], xd_ps)

                    shared_gu(2, pgsh)

                # weighted combine one-hots, built directly transposed
                DpT = {}
                for e in range(EPC):
                    DpT[e] = persist.tile([128, T], BF16, name=f"DpT{e}")
                    nc.vector.tensor_scalar(
                        DpT[e], iota_row, idw0[e][:, 0:1], idw0[e][:, 1:2],
                        op0=ALU.is_equal, op1=ALU.mult)
                if csz1 > 0:
                    # spill combine rows live at partitions 96.. of the
                    # third shared tile (hsh js2 / swd js2)
                    nc.vector.tensor_scalar(
                        hsh[ds(96, csz1), 2],
                        iota_row[ds(96, csz1)],
                        iw1[ds(96, csz1), 0:1],
                        iw1[ds(96, csz1), 1:2],
                        op0=ALU.is_equal, op1=ALU.mult)


            # ============ experts: gate_up -> silu*up -> down ============
            # swd loads now (row range 96.. of its third tile is later
            # overwritten by the spill-down evacuation)
            swd_sb = shp.tile([128, JSH, H], BF16, tag="sw", name="swd")
            nc.sync.dma_start(swd_sb, swd_d[:])
            h_sb = {}
            y0 = {}
            for e in range(EPC):
                Ce = caps[e]
                load_wd(e)
                h_sb[e] = persist.tile([128, IT, Ce], BF16, name=f"h{e}")
                with tc.tile_pool(name=f"pgu{e}", space="PSUM", bufs=4) as pgu:
                    for j in range(IT):
                        if (e, j) in wgu_tiles:
                            wg = wgu_tiles.pop((e, j))
                        else:
                            wg = wpool.tile([128, 2, H], F8E3, tag="wgu")
                            nc.sync.dma_start(wg, wgu_d[e, j])
                        ps_g = pgu.tile([128, Ce], F32, tag="gu")
                        ps_u = pgu.tile([128, Ce], F32, tag="gu")
                        for hc in range(HT):
                            nc.tensor.matmul(
                                ps_g, wg[:, 0, ts(hc, 128)],
                                xd[:, hc, ds(off[e], Ce)],
                                start=(hc == 0), stop=(hc == HT - 1))
                        for hc in range(HT):
                            nc.tensor.matmul(
                                ps_u, wg[:, 1, ts(hc, 128)],
                                xd[:, hc, ds(off[e], Ce)],
                                start=(hc == 0), stop=(hc == HT - 1))
                        # true g = ps_g * s_gu; h kept scaled by 1/s_gu^2
                        sg = rsb.tile([128, Ce], BF16, tag="sg")
                        nc.scalar.activation(sg, ps_g, ACTF.Sigmoid,
                                             scale=float(s_gu))
                        sg2 = rsb.tile([128, Ce], BF16, tag="sg2")
                        nc.vector.tensor_mul(sg2, sg, ps_g)
                        nc.vector.tensor_mul(h_sb[e][:, j], sg2, ps_u)
                        if e + 1 < EPC and j >= IT - 2:
                            load_wgu(e + 1, j - (IT - 2))

                # down ct0: full-array MMs, N=512
                y0[e] = persist.tile([128, H], BF16, name=f"y0{e}")
                with tc.tile_pool(name=f"py{e}", space="PSUM", bufs=1) as py:
                    ps_y = [py.tile([128, 512], F32, tag=f"py{hk}",
                                    name=f"psy{hk}") for hk in range(HK)]
                    for ic in range(IT):
                        for hk in range(HK):
                            nc.tensor.matmul(
                                ps_y[hk], h_sb[e][:, ic, 0:128],
                                wd_sb[e][:, ic, ts(hk, 512)],
                                start=(ic == 0), stop=(ic == IT - 1))
                    for hk in range(HK):
                        if hk % 2 == 0:
                            nc.vector.tensor_scalar(
                                y0[e][:, ts(hk, 512)], ps_y[hk], SC_Y, None,
                                op0=ALU.mult)
                        else:
                            nc.scalar.activation(
                                y0[e][:, ts(hk, 512)], ps_y[hk], ACTF.Copy,
                                scale=SC_Y)

                # down spill slots (only expert 0 can spill): psum and
                # evac live at partitions 96.. so they land in the third
                # shared tile rows of swd
                if spills[e] > 0:
                    with tc.tile_pool(name=f"pm{e}", space="PSUM",
                                      bufs=1) as pm:
                        ps_m = [pm.tile([128, 512], F32, tag=f"pm{hk}",
                                        name=f"pm{e}{hk}")
                                for hk in range(HK)]
                        sl = ds(96, spills[e])
                        for ic in range(IT):
                            for hk in range(HK):
                                nc.tensor.matmul(
                                    ps_m[hk][sl],
                                    h_sb[e][:, ic, ds(128, spills[e])],
                                    wd_sb[e][:, ic, ts(hk, 512)],
                                    start=(ic == 0), stop=(ic == IT - 1),
                                    tile_position=(0, 96))
                        for hk in range(HK):
                            if hk % 2 == 0:
                                nc.vector.tensor_scalar(
                                    swd_sb[sl, 2, ts(hk, 512)], ps_m[hk][sl],
                                    SC_Y, None, op0=ALU.mult)
                            else:
                                nc.scalar.activation(
                                    swd_sb[sl, 2, ts(hk, 512)], ps_m[hk][sl],
                                    ACTF.Copy, scale=SC_Y)

            # ------------- combine: routed (weighted) + shared -------------
            with (
                tc.tile_pool(name="po", space="PSUM", bufs=2) as po,
                tc.tile_pool(name="osb", bufs=4) as osb,
            ):
                chain = [("sh", js) for js in range(JSH)]
                chain += [("e0", e) for e in range(EPC)]
                for tt in range(TT):
                    for hk in range(HK):
                        ps_o = po.tile([128, 512], F32, tag="o", name="ps_o")
                        for n, (a, b) in enumerate(chain):
                            st, sp = (n == 0), (n == len(chain) - 1)
                            if a == "sh":
                                nc.tensor.matmul(
                                    ps_o, hsh[:, b, ts(tt, 128)],
                                    swd_sb[:, b, ts(hk, 512)],
                                    start=st, stop=sp)
                            else:
                                nc.tensor.matmul(
                                    ps_o, DpT[b][:, ts(tt, 128)],
                                    y0[b][:, ts(hk, 512)], start=st, stop=sp)
                        o_sb = osb.tile([128, 512], BF16, tag="o")
                        if hk % 2 == 0:
                            nc.vector.tensor_copy(o_sb, ps_o)
                        else:
                            nc.scalar.copy(o_sb, ps_o)
                        nc.sync.dma_start(
                            out_d[ts(tt, 128), ts(hk, 512)], o_sb)
    nc.compile()
    return nc


def _route(x64: np.ndarray, gate_w64: np.ndarray):
    """fp64 routing + near-tie margin; pairs the i-th biggest-count expert
    with the i-th smallest on each core. Returns (C0, C1, pairs)."""
    logits = x64 @ gate_w64.T
    order = np.argsort(-logits, axis=1)
    counts = np.bincount(order[:, :K].ravel(), minlength=E).astype(np.int64)
    srt = np.sort(logits, axis=1)
    gap = srt[:, E - K] - srt[:, E - K - 1]
    for t in np.nonzero(gap < 2e-3)[0]:
        counts[order[t, K - 1]] += 1
        counts[order[t, K]] += 1
    by_count = sorted(range(E), key=lambda e: -counts[e])
    pairs = [(by_count[c], by_count[E - 1 - c]) for c in range(NCORES)]
    r8 = lambda v: ((int(v) + 7) // 8) * 8
    C0 = min(256, max(128, r8(max(counts[a] for a, _ in pairs))))
    C1 = min(C0, max(64, r8(max(counts[b] for _, b in pairs))))
    return C0, C1, pairs


def _lhsT_tiles(Wt: np.ndarray, col0: int, ncol: int = 128) -> np.ndarray:
    """Wt: [H, cols]. Returns [128, HT*ncol] with (p, ncol*hc + c) =
    Wt[128*hc + p, col0 + c] - lhsT chunk layout over the 16 h-chunks."""
    blk = Wt[:, col0:col0 + ncol].reshape(HT, 128, ncol)
    return np.ascontiguousarray(blk.transpose(1, 0, 2)).reshape(128, HT * ncol)


_BUILD_CACHE = {}


def prepare(hidden_states, gate_w, w_gate_up, w_down, shared_gate_up,
            shared_down):
    """Host-side sharding/layout/quantization prep.

    Returns (C0, C1, s_gu, s_d, in_maps)."""
    x = np.ascontiguousarray(np.asarray(hidden_states, dtype=np.float32))
    gate_w = np.asarray(gate_w, dtype=np.float32)
    w_gate_up = np.asarray(w_gate_up, dtype=np.float32)
    w_down = np.asarray(w_down, dtype=np.float32)
    shared_gate_up = np.asarray(shared_gate_up, dtype=np.float32)
    shared_down = np.asarray(shared_down, dtype=np.float32)

    C0, C1, pairs = _route(x.astype(np.float64), gate_w.astype(np.float64))
    s_gu = float(np.abs(w_gate_up).max()) / 15.5
    s_d = float(np.abs(w_down).max()) / 15.5

    # common (replicated) activations + constants
    xt_r = np.ascontiguousarray(
        x.T.reshape(HT, 128, T).transpose(1, 0, 2))     # [128, HT, T] f32
    xt_bf = xt_r.astype(BF)
    xt_lo = (xt_r - xt_bf.astype(np.float32)).astype(BF)
    x_bf = np.ascontiguousarray(
        x.reshape(TT, 128, H).transpose(1, 0, 2)).astype(BF)
    csts = np.zeros((128, T + TT + 128), np.float32)
    csts[:, 0:T] = np.arange(T, dtype=np.float32)[None, :]
    csts[:, T:T + TT] = (np.arange(128, dtype=np.float32)[:, None]
                         + 128.0 * np.arange(TT, dtype=np.float32)[None, :])
    csts[:, T + TT:] = np.eye(128, dtype=np.float32)
    tri = np.ascontiguousarray(
        np.triu(np.ones((T, T), np.float32), 1)
        .reshape(TT, 128, T).transpose(1, 0, 2)).astype(BF)

    # shared expert: per-core shard of gate/up/down (352 channels each)
    sg_T = np.ascontiguousarray(shared_gate_up[:IS].T)
    su_T = np.ascontiguousarray(shared_gate_up[IS:].T)
    sd_T = np.ascontiguousarray(shared_down.T)

    in_maps = []
    for c in range(NCORES):
        own = list(pairs[c])                     # big-count expert first
        perm = own + [e for e in range(E) if e not in own]
        gperm = gate_w[perm].T.reshape(HT, 128, E).transpose(1, 0, 2)
        gw_hl = np.empty((128, 2, HT, E), BF)
        gw_hl[:, 0] = gperm.astype(BF)
        gw_hl[:, 1] = (gperm - gw_hl[:, 0].astype(np.float32)).astype(BF)

        wgu8 = np.empty((EPC, IT, 128, 2, H), E3)
        wd8 = np.empty((EPC, 128, IT, H), E3)
        for el, eg in enumerate(own):
            Wt8f = (w_gate_up[eg].T / s_gu).astype(E3).astype(np.float32)
            for j in range(IT):
                wgu8[el, j, :, 0, :] = _lhsT_tiles(Wt8f, j * 128).astype(E3)
                wgu8[el, j, :, 1, :] = _lhsT_tiles(Wt8f, I + j * 128).astype(E3)
            Wd8 = (w_down[eg].T / s_d).astype(E3)            # [I, H]
            wd8[el] = Wd8.reshape(IT, 128, H).transpose(1, 0, 2)

        base = c * ISC
        swgu = np.empty((128, 2, 2, H), BF)
        for js in range(2):
            swgu[:, js, 0, :] = _lhsT_tiles(sg_T, base + js * 128).astype(BF)
            swgu[:, js, 1, :] = _lhsT_tiles(su_T, base + js * 128).astype(BF)
        swgu2 = np.empty((128, 2, HT * JS2), BF)
        swgu2[:, 0] = _lhsT_tiles(sg_T, base + 256, JS2).astype(BF)
        swgu2[:, 1] = _lhsT_tiles(su_T, base + 256, JS2).astype(BF)
        swgu2 = swgu2.reshape(128, 2, HT, JS2)
        swd = np.zeros((128, JSH, H), BF)
        swd[:, 0] = sd_T[base:base + 128].astype(BF)
        swd[:, 1] = sd_T[base + 128:base + 256].astype(BF)
        swd[:JS2, 2] = sd_T[base + 256:base + ISC].astype(BF)

        in_maps.append({
            "gw_hl": gw_hl, "xt_lo": xt_lo, "xt_bf": xt_bf, "x_bf": x_bf,
            "csts": csts, "tri": tri,
            "wgu8": wgu8, "wd8": wd8, "swgu": swgu, "swgu2": swgu2,
            "swd": swd,
        })
    return C0, C1, s_gu, s_d, in_maps


def kernel(**inputs):
    C0, C1, s_gu, s_d, in_maps = prepare(**inputs)
    key = (C0, C1, round(s_gu, 9), round(s_d, 9))
    if key not in _BUILD_CACHE:
        _BUILD_CACHE[key] = build_nc(C0, C1, s_gu, s_d)
    nc = _BUILD_CACHE[key]

    res = run_bass_kernel_spmd(nc, in_maps, core_ids=list(range(NCORES)))
    out = np.zeros((T, H), np.float32)
    for r in res.results:
        out += r["out"].astype(np.float32)
    return out
